# revision 1
# baseline (speedup 1.0000x reference)
"""Trainium2 Bass kernel for MultiHeadSelfAttention + RoPE (B=2, S=2048, D=1024, H=16).

Sharding: 8 cores = 2 (batch) x 4 (head-groups of 4 heads).
Per-core: project q/k/v for its 4 heads (fp32r matmuls), apply RoPE on DVE,
causal attention with transposed scores (S^T[j,i]) so softmax-exp runs on the
Scalar engine straight out of PSUM, attn@V with a ones-augmented V (M=65) so
the softmax denominator falls out of the same matmuls, per-query normalize,
then the sliced output projection. Host sums the 4 head-group partials per batch.
"""
import math
import os
import sys
_SKIP = set(os.environ.get('K_SKIP', '').split(','))

import numpy as np

for _p in ("/opt/trn_rl_repo", "/root/.axon_site/_ro/trn_rl_repo"):
    if os.path.isdir(_p) and _p not in sys.path:
        sys.path.insert(0, _p)

import concourse.bacc as bacc
import concourse.bass as bass
import concourse.tile as tile
from concourse import mybir
from concourse import bass_utils

B, S, D = 2, 2048, 1024
H = 16
NCORE = 8
HPC = 4                 # heads per core
E = HPC * 64            # 256: per-core e-width
DK = 64
THETA = 10000.0
CH = 512                # query chunk width
NCH = S // CH           # 4
NJT = S // 128          # 16 j-tiles
F32 = mybir.dt.float32
F32R = mybir.dt.float32r

_programs = {}
LAST_RESULT = None


def _build(share_x: bool, loop_n: int = 0, bench_internal: bool = False):
    nc = bacc.Bacc("TRN2", target_bir_lowering=False)
    kind_in = "Internal" if bench_internal else "ExternalInput"
    xt_qk = nc.dram_tensor("xt_qk", [D, S], F32R, kind=kind_in)
    xt_v = nc.dram_tensor("xt_v", [D, S], F32R, kind=kind_in)
    wqt = nc.dram_tensor("wqt", [D, E], F32R, kind=kind_in)
    wkt = nc.dram_tensor("wkt", [D, E], F32R, kind=kind_in)
    wvt = nc.dram_tensor("wvt", [D, E], F32R, kind=kind_in)
    ot = nc.dram_tensor("ot", [E, D], F32R, kind=kind_in)
    cc = nc.dram_tensor("cc", [128, S], F32, kind=kind_in)
    ss = nc.dram_tensor("ss", [128, S], F32, kind=kind_in)
    tri = nc.dram_tensor("tri", [128, 128], F32, kind=kind_in)
    ones = nc.dram_tensor("ones", [128, NJT * HPC], F32R, kind=kind_in)
    out_t = nc.dram_tensor(
        "out_t", [D, S], F32, kind="Internal" if bench_internal else "ExternalOutput")
    tick = nc.dram_tensor("tick", [16, 16], F32, kind="ExternalOutput") \
        if bench_internal else None

    Exp = mybir.ActivationFunctionType.Exp
    inv_sqrt_dk = 1.0 / math.sqrt(DK)

    with tile.TileContext(nc) as tc:
        with tc.tile_pool(name="persist", bufs=1) as persist:
            ot_sb = persist.tile([128, 2, D], F32R, tag="ot")
            tri_sb = persist.tile([128, 128], F32, tag="tri")
            warm = persist.tile([1, 1], F32, tag="warm")
            nc.vector.memset(warm[:], 0.0)
            nc.scalar.activation(warm[:], warm[:], Exp, scale=1.0)
            qr_sb = persist.tile([128, 2 * S], F32R, tag="qr")
            kr_sb = persist.tile([128, 2 * S], F32R, tag="kr")
            vaug = persist.tile([128, NJT, HPC * 65], F32R, tag="vaug")

            # ---------------- phase 1+2: projections + rope ----------------
            with tc.tile_pool(name="wx", bufs=1) as wx, \
                 tc.tile_pool(name="tab", bufs=1) as tab, \
                 tc.tile_pool(name="rope", bufs=2) as rope, \
                 tc.tile_pool(name="psp1", bufs=6, space="PSUM") as psproj:
                x_sb = wx.tile([128, 8, S], F32R, tag="x")
                wq_sb = wx.tile([128, 8, E], F32R, tag="wq")
                wk_sb = wx.tile([128, 8, E], F32R, tag="wk")
                wv_sb = wx.tile([128, 8, E], F32R, tag="wv")
                cc_sb = tab.tile([128, S], F32, tag="cc")
                ss_sb = tab.tile([128, S], F32, tag="ss")
                # weights/tables on the gpsimd (SWDGE) queue, x on sync (HWDGE):
                # both streams run in parallel and the first matmul group only
                # waits for wq + x.
                nc.gpsimd.dma_start(wq_sb[:], wqt[:, :].rearrange("(k p) e -> p k e", p=128))
                nc.gpsimd.dma_start(wk_sb[:], wkt[:, :].rearrange("(k p) e -> p k e", p=128))
                nc.gpsimd.dma_start(cc_sb[:], cc[:, :])
                nc.gpsimd.dma_start(ss_sb[:], ss[:, :])
                x_re = xt_qk[:, :].rearrange("(k p) s -> p k s", p=128)
                for kt in range(8):
                    nc.sync.dma_start(x_sb[:, kt], x_re[:, kt])

                # q/k projections, rope fused per psum tile (et-outer so
                # head-pair 0 finishes first and attention can start early)
                for et in range(2):
                    for w_sb, dst in ((wq_sb, qr_sb), (wk_sb, kr_sb)):
                        for sc in range(NCH):
                            pp = psproj.tile([128, CH], F32, tag="pp")
                            for kt in range(8):
                                nc.tensor.matmul(
                                    pp[:],
                                    w_sb[:, kt, et * 128:(et + 1) * 128],
                                    x_sb[:, kt, sc * CH:(sc + 1) * CH],
                                    start=(kt == 0), stop=(kt == 7))
                            p_sb = rope.tile([128, CH], F32, tag="p")
                            nc.vector.tensor_copy(p_sb[:], pp[:])
                            # pairwise 32-block partition swap (evens<->odds),
                            # split between scalar and gpsimd engines
                            psw = rope.tile([128, CH], F32, tag="psw")
                            for blk in range(4):
                                sb_ = 32 * (blk ^ 1)
                                eng = nc.scalar.copy if blk % 2 == 0 else nc.gpsimd.tensor_copy
                                eng(psw[32 * blk:32 * blk + 32, :],
                                    p_sb[sb_:sb_ + 32, :])
                            t_sb = rope.tile([128, CH], F32, tag="t")
                            nc.vector.tensor_mul(t_sb[:], p_sb[:], cc_sb[:, sc * CH:(sc + 1) * CH])
                            nc.vector.tensor_mul(psw[:], psw[:], ss_sb[:, sc * CH:(sc + 1) * CH])
                            nc.vector.tensor_add(
                                dst[:, et * S + sc * CH: et * S + (sc + 1) * CH],
                                t_sb[:], psw[:])

                nc.gpsimd.dma_start(wv_sb[:], wvt[:, :].rearrange("(k p) e -> p k e", p=128))
                nc.gpsimd.dma_start(tri_sb[:], tri[:, :])
                nc.gpsimd.dma_start(ot_sb[:], ot[:, :].rearrange("(t p) m -> p t m", p=128))
                # v projection into ones-augmented layout
                if not share_x:
                    xv_re = xt_v[:, :].rearrange("(k p) s -> p k s", p=128)
                    for kt in range(8):
                        nc.sync.dma_start(x_sb[:, kt], xv_re[:, kt])
                ones_view = vaug[:].rearrange("p j (h c) -> p j h c", c=65)[:, :, :, 64:65]
                nc.sync.dma_start(
                    ones_view,
                    ones[:, :].rearrange("p (j h c) -> p j h c", j=NJT, h=HPC))
                for st in range(NJT):
                    pv = psproj.tile([128, CH], F32, tag="pp")
                    for kt in range(8):
                        nc.tensor.matmul(
                            pv[:, 0:E],
                            x_sb[:, kt, st * 128:(st + 1) * 128],
                            wv_sb[:, kt],
                            start=(kt == 0), stop=(kt == 7))
                    dst = vaug[:, st].rearrange("p (h c) -> p h c", c=65)[:, :, 0:64]
                    nc.scalar.copy(dst, pv[:, 0:E].rearrange("p (h c) -> p h c", c=64))

            # ---------------- phase 3: attention ----------------
            with tc.tile_pool(name="pss", bufs=2, space="PSUM") as pss, \
                 tc.tile_pool(name="psu", bufs=2, space="PSUM") as psu, \
                 tc.tile_pool(name="psrb", bufs=1, space="PSUM") as psrb, \
                 tc.tile_pool(name="pso", bufs=1, space="PSUM") as psproj, \
                 tc.tile_pool(name="att", bufs=4) as att, \
                 tc.tile_pool(name="small", bufs=3) as small, \
                 tc.tile_pool(name="mhap", bufs=1) as mhap, \
                 tc.tile_pool(name="outp", bufs=4) as outp:
                ones_sb = mhap.tile([1, 64], F32R, tag="ones1")
                nc.sync.dma_start(ones_sb[:], ones[0:1, 0:64])
                mha_0 = mhap.tile([128, S], F32R, tag="mha0")
                mha_1 = mhap.tile([128, S], F32R, tag="mha1")
                mha01 = [mha_0, mha_1]
                import contextlib
                loop_ctx = tc.For_i(0, loop_n, 1) \
                    if (loop_n and os.environ.get("K_LOOP_SITE") != "proj") \
                    else contextlib.nullcontext()
                with loop_ctx:
                  prev_c = None
                  prev_norm = None
                  pending_oproj = []
                  def emit_oproj_mt(ic, mt):
                      po = psproj.tile([128, CH], F32, tag="pp", name=f"po{ic}_{mt}")
                      for vt in range(2):
                          nc.tensor.matmul(
                              po[:],
                              ot_sb[:, vt, mt * 128:(mt + 1) * 128],
                              mha01[vt][:, ic * CH:(ic + 1) * CH],
                              start=(vt == 0), stop=(vt == 1))
                      so = outp.tile([128, CH], F32, tag="so", name=f"so{ic}_{mt}")
                      nc.vector.tensor_copy(so[:], po[:])
                      if "odma" not in _SKIP:
                          nc.sync.dma_start(
                              out_t[mt * 128:(mt + 1) * 128, ic * CH:(ic + 1) * CH],
                              so[:])
                  for c in range(NCH):
                      for hp in range(2):
                          base = hp * S
                          mha = mha01[hp]
                          u_ab = [psu.tile([65, CH], F32, tag="u", name=f"u{c}{hp}{hb}")
                                  for hb in range(2)]
                          njt = 4 * c + 4
                          for jt in range(njt):
                              s_ab = pss.tile([128, 2 * CH], F32, tag="s")
                              j0 = base + jt * 128
                              i0 = base + c * CH
                              nc.tensor.matmul(
                                  s_ab[:, 0:CH],
                                  kr_sb[0:64, j0:j0 + 128],
                                  qr_sb[0:64, i0:i0 + CH],
                                  start=True, stop=True, tile_position=(0, 0))
                              nc.tensor.matmul(
                                  s_ab[:, CH:2 * CH],
                                  kr_sb[64:128, j0:j0 + 128],
                                  qr_sb[64:128, i0:i0 + CH],
                                  start=True, stop=True, tile_position=(64, 0))
                              off = 128 * (jt - 4 * c) if jt >= 4 * c else 0
                              e_ab = att.tile([128, 2 * CH], F32R, tag="e")
                              s_v = s_ab[:].rearrange("p (h i) -> p h i", h=2)[:, :, off:]
                              e_v = e_ab[:].rearrange("p (h i) -> p h i", h=2)[:, :, off:]
                              nc.scalar.activation(e_v, s_v, Exp, scale=inv_sqrt_dk)
                              if jt >= 4 * c and "mask" not in _SKIP:
                                  em = e_ab[:].rearrange(
                                      "p (h i) -> p h i", h=2)[:, :, off:off + 128]
                                  t_ = tri_sb[:]
                                  tri_b = bass.AP(t_.tensor, t_.offset,
                                                  [t_.ap[0], [0, 2], t_.ap[1]])
                                  nc.vector.tensor_mul(em, em, tri_b)
                              for hb in range(2):
                                  nc.tensor.matmul(
                                      u_ab[hb][0:65, off:CH],
                                      vaug[:, jt, (2 * hp + hb) * 65:(2 * hp + hb + 1) * 65],
                                      e_ab[:, hb * CH + off:(hb + 1) * CH],
                                      start=(jt == 0), stop=(jt == njt - 1))
                          # evacuate U so the psum accumulator frees quickly,
                          # then normalize: mha = U[0:64] * (1 / U[64])
                          if "norm" in _SKIP:
                              continue
                          ucps = []
                          for hb in range(2):
                              ucp = small.tile([65, CH], F32, tag="ucp",
                                               name=f"ucp{c}{hp}{hb}")
                              if hb == 0:
                                  nc.vector.tensor_copy(ucp[:], u_ab[hb][0:65, :])
                              else:
                                  nc.scalar.copy(ucp[:], u_ab[hb][0:65, :])
                              ucps.append(ucp)
                          def emit_norm(ucps, c, hp):
                              mha_ = mha01[hp]
                              for hb in range(2):
                                  rec = small.tile([1, CH], F32R, tag="rec")
                                  with nc.allow_low_precision(reason="f32r 4-byte tag"):
                                      nc.vector.reciprocal(rec[:], ucps[hb][64:65, :])
                                  rb = psrb.tile([64, CH], F32, tag="rb")
                                  nc.tensor.matmul(rb[0:64, :], ones_sb[0:1, 0:64],
                                                   rec[0:1, :], start=True, stop=True)
                                  if hb == 0:
                                      nc.vector.tensor_mul(
                                          mha_[0:64, c * CH:(c + 1) * CH],
                                          ucps[hb][0:64, :], rb[0:64, :])
                                  else:
                                      tmpb = small.tile([64, CH], F32R, tag="tmpb")
                                      nc.vector.tensor_mul(tmpb[:], ucps[hb][0:64, :],
                                                           rb[0:64, :])
                                      nc.gpsimd.tensor_copy(
                                          mha_[64:128, c * CH:(c + 1) * CH], tmpb[:])
                          emit_norm(ucps, c, hp)

                      # ---- output projection, deferred and sprinkled one
                      # mt-slice per jt of the following chunk ----
                      if prev_c is not None and "oproj" not in _SKIP:
                          for mt in range(8):
                              emit_oproj_mt(prev_c, mt)
                      prev_c = c
                  if "oproj" not in _SKIP:
                      for mt in range(8):
                          emit_oproj_mt(prev_c, mt)
            if tick is not None:
                nc.sync.dma_start(tick[:, :], tri_sb[0:16, 0:16])
    nc.compile()
    return nc


def _get_program(share_x: bool):
    if share_x not in _programs:
        _programs[share_x] = _build(share_x)
    return _programs[share_x]


def kernel(x, token_positions, q_weight, k_weight, v_weight, o_weight):
    global LAST_RESULT
    x = np.ascontiguousarray(np.asarray(x), dtype=np.float32)
    pos = np.asarray(token_positions)
    q_weight = np.asarray(q_weight, dtype=np.float32)
    k_weight = np.asarray(k_weight, dtype=np.float32)
    v_weight = np.asarray(v_weight, dtype=np.float32)
    o_weight = np.asarray(o_weight, dtype=np.float32)

    share = bool(np.array_equal(pos, np.arange(S, dtype=pos.dtype)))
    nc = _get_program(share)

    # rope tables in the [4x(evens,odds-swapped)] block layout
    inv = THETA ** (-np.arange(DK // 2, dtype=np.float32) * 2.0 / DK)
    ang = pos.astype(np.float32)[:, None] * inv[None, :]        # (S, 32)
    C = np.cos(ang).T.astype(np.float32)                        # (32, S)
    S_ = np.sin(ang).T.astype(np.float32)
    CC = np.tile(C, (4, 1)).astype(np.float32)                  # (128, S)
    SS = np.concatenate([-S_, S_, -S_, S_], axis=0).astype(np.float32)
    ii = np.arange(128)
    tri = (ii[:, None] <= ii[None, :]).astype(np.float32)

    in_maps = []
    for core in range(NCORE):
        b, hg = divmod(core, 4)
        h0 = HPC * hg
        perm = []
        for h in range(h0, h0 + HPC):
            perm += list(range(64 * h, 64 * h + 64, 2))
            perm += list(range(64 * h + 1, 64 * h + 64, 2))
        xb = x[b]
        xTv = np.ascontiguousarray(xb.T)
        xTqk = xTv if share else np.ascontiguousarray(xb[pos].T)
        ecols = slice(64 * h0, 64 * h0 + E)
        in_maps.append({
            "xt_qk": xTqk,
            "xt_v": xTv,
            "wqt": np.ascontiguousarray(q_weight[perm].T),
            "wkt": np.ascontiguousarray(k_weight[perm].T),
            "wvt": np.ascontiguousarray(v_weight[ecols].T),
            "ot": np.ascontiguousarray(o_weight[:, ecols].T),
            "cc": CC,
            "ss": SS,
            "tri": tri,
            "ones": np.ones((128, NJT * HPC), np.float32),
        })

    res = bass_utils.run_bass_kernel_spmd(nc, in_maps, core_ids=list(range(NCORE)))
    LAST_RESULT = res
    out = np.zeros((B, S, D), np.float32)
    for core in range(NCORE):
        out[core // 4] += res.results[core]["out_t"].T
    return out

